# revision 5
# baseline (speedup 1.0000x reference)
"""RGCN basis-decomposition message-passing layer on 8 Trainium2 NeuronCores.

Reference semantics (per edge e: src -> dst with relation r, W[r] = comp[r] @ basis):
    msg[e]  = x[src[e]] @ W[r[e]] / count[dst[e], r[e]]
    agg     = segment_sum(msg, dst)
    out     = agg + x @ root + bias
    returns (out, edge_attr)          # edge_update is identity

Distribution: destination-sharded.  Core c owns dst nodes [c*N/8, (c+1)*N/8);
its edges are exactly the edges pointing into that range, so each core produces
a disjoint slice of the output and no collective is needed.

Device algorithm (aggregate-then-transform, matmul-based scatter):
  * edges sorted by local dst, bucketed into 64-dst windows; each window gets a
    fixed number of 128-edge chunk slots per source-half plane (the SPMD
    program is shared by all cores, so the slot grid is uniform; unused slots
    carry index 0 and weight 0 and contribute nothing).
  * source rows are fetched with gpsimd dma_gather (int16 indices, so x is
    split into a low and a high half table; slots are segregated by plane).
  * per chunk: a selection matrix S[e, j] = w_e * (iota[j] == dstloc_e*R + r_e)
    built by one dual-op tensor_scalar (j = 64 dst x 8 relations = 512), and
    one PE matmul accumulating Zt[i, j] += Xg^T @ S into a PSUM bank.
  * per 128-dst group (2 windows): outT[o, d] = sum_r W_r^T @ Zt_r
    + root^T @ xT + bias (rank-1), accumulated in one PSUM group.
  * output written transposed ([128 out x NSH dst]); host transposes back.

Counts (mean normalization) are index-only metadata; the host derives
w_e = 1/count[dst_e, r_e] from the same sort used to build the slot layout.
"""

import numpy as np

_PROGRAM_CACHE = {}

NIDX_CALL = 1024          # dma_gather indices per call (HW limit)
COLS_CALL = NIDX_CALL // 128


class _Geom:
    def __init__(self, n_nodes, n_rel, c_lo, c_hi, cores=8, win=64, grp=128,
                 chunk=128):
        assert n_nodes % cores == 0
        self.N = n_nodes
        self.R = n_rel
        self.CORES = cores
        self.WIN = win                      # dst nodes per window
        self.GRP = grp                      # dst nodes per group (psum out tile)
        self.CHUNK = chunk                  # edges per chunk (= K of matmul)
        self.J = win * n_rel                # one-hot width (<=512)
        assert self.J <= 512
        self.NSH_REAL = n_nodes // cores
        self.NG = -(-self.NSH_REAL // grp)
        self.NSH = self.NG * grp
        self.NW = self.NSH // win
        self.WPG = grp // win
        self.SPLIT = n_nodes // 2           # lo plane: src < SPLIT
        self.C_LO = c_lo                    # chunk slots per window, lo plane
        self.C_HI = c_hi
        # plane regions padded to whole gather calls
        self.LO_COLS = -(-self.NW * c_lo // COLS_CALL) * COLS_CALL
        self.HI_COLS = -(-self.NW * c_hi // COLS_CALL) * COLS_CALL
        self.SLOTS = self.LO_COLS + self.HI_COLS
        self.NGATH = self.SLOTS // COLS_CALL

    def slot(self, w, plane, k):
        if plane == 0:
            return w * self.C_LO + k
        return self.LO_COLS + w * self.C_HI + k

    def key(self):
        return (self.N, self.R, self.CORES, self.WIN, self.GRP, self.CHUNK,
                self.C_LO, self.C_HI)


# ---------------------------------------------------------------------------
# host-side sharding / metadata prep
# ---------------------------------------------------------------------------

def _prep_core(g, core, src, dst, et):
    lo = core * g.NSH_REAL
    hi = lo + g.NSH_REAL
    sel = np.nonzero((dst >= lo) & (dst < hi))[0]
    dloc = (dst[sel] - lo).astype(np.int64)
    s_src = src[sel].astype(np.int64)
    s_et = et[sel].astype(np.int64)

    plane = (s_src >= g.SPLIT).astype(np.int64)
    order = np.lexsort((dloc, plane, dloc // g.WIN))
    dloc, s_src, s_et, plane = (dloc[order], s_src[order], s_et[order],
                                plane[order])

    ckey = dloc * g.R + s_et
    cnt = np.bincount(ckey, minlength=g.NSH * g.R)
    w = 1.0 / cnt[ckey]

    wid = dloc // g.WIN
    # rank within (window, plane) run
    seg = wid * 2 + plane
    seg_starts = np.searchsorted(seg, np.arange(g.NW * 2))
    rank = np.arange(len(dloc)) - seg_starts[seg]
    kslot = rank // g.CHUNK
    c_lo_need = int(kslot[plane == 0].max()) + 1 if (plane == 0).any() else 0
    c_hi_need = int(kslot[plane == 1].max()) + 1 if (plane == 1).any() else 0
    if c_lo_need > g.C_LO or c_hi_need > g.C_HI:
        raise OverflowError((c_lo_need, c_hi_need))

    slot = np.where(plane == 0,
                    wid * g.C_LO + kslot,
                    g.LO_COLS + wid * g.C_HI + kslot)
    p = rank % g.CHUNK

    gidx = np.zeros((g.CHUNK, g.SLOTS), dtype=np.int16)
    key = np.full((g.CHUNK, g.SLOTS), 1000.0, dtype=np.float32)
    wv = np.zeros((g.CHUNK, g.SLOTS), dtype=np.float32)
    local_src = np.where(plane == 0, s_src, s_src - g.SPLIT)
    gidx[p, slot] = local_src.astype(np.int16)
    key[p, slot] = ((dloc - wid * g.WIN) * g.R + s_et).astype(np.float32)
    wv[p, slot] = w.astype(np.float32)

    # wrapped int16 index layout for dma_gather: stream position i ->
    # partition i % 16, column i // 16; replicated across the 8 q7 cores.
    flat = gidx.T.reshape(-1)                        # stream order (col-major)
    wrapped = flat.reshape(-1, 16).T                 # [16, SLOTS*8]
    gidx_dev = np.ascontiguousarray(np.tile(wrapped, (8, 1)))
    return gidx_dev, key, wv


def _host_prep(g, x, edge_index, edge_type, basis, comp, root, bias):
    src = np.asarray(edge_index[0])
    dst = np.asarray(edge_index[1])
    et = np.asarray(edge_type)
    x = np.ascontiguousarray(np.asarray(x, dtype=np.float32))
    IN = x.shape[1]
    OUTD = root.shape[1]

    W = np.einsum("rb,bio->rio", np.asarray(comp, np.float32),
                  np.asarray(basis, np.float32))
    Wt = np.ascontiguousarray(W.transpose(1, 0, 2).reshape(IN, g.R * OUTD))
    iota = np.ascontiguousarray(
        np.tile(np.arange(g.J, dtype=np.float32), (g.CHUNK, 1)))
    root_f = np.ascontiguousarray(np.asarray(root, np.float32))
    bias_f = np.ascontiguousarray(np.asarray(bias, np.float32).reshape(1, OUTD))
    ones = np.ones((1, g.GRP), dtype=np.float32)
    x_lo = np.ascontiguousarray(x[:g.SPLIT])
    x_hi = np.ascontiguousarray(x[g.SPLIT:])

    in_maps = []
    for c in range(g.CORES):
        gidx_dev, key, wv = _prep_core(g, c, src, dst, et)
        lo = c * g.NSH_REAL
        xT = np.zeros((IN, g.NSH), dtype=np.float32)
        xT[:, :g.NSH_REAL] = x[lo:lo + g.NSH_REAL].T
        in_maps.append({
            "x_lo": x_lo, "x_hi": x_hi,
            "xT": np.ascontiguousarray(xT),
            "gidx": gidx_dev, "key": key, "wv": wv,
            "iota": iota, "Wt": Wt, "root": root_f,
            "bias": bias_f, "ones": ones,
        })
    return in_maps


# ---------------------------------------------------------------------------
# device program
# ---------------------------------------------------------------------------

def _build_program(g, IN, OUTD):
    import concourse.bacc as bacc
    import concourse.mybir as mybir
    from concourse.tile import TileContext

    f32 = mybir.dt.float32
    i16 = mybir.dt.int16
    nc = bacc.Bacc(None, target_bir_lowering=False)

    xlo_d = nc.declare_dram_parameter("x_lo", [g.SPLIT, IN], f32, isOutput=False)
    xhi_d = nc.declare_dram_parameter("x_hi", [g.N - g.SPLIT, IN], f32, isOutput=False)
    xT_d = nc.declare_dram_parameter("xT", [IN, g.NSH], f32, isOutput=False)
    gidx_d = nc.declare_dram_parameter("gidx", [128, g.SLOTS * g.CHUNK // 16], i16, isOutput=False)
    key_d = nc.declare_dram_parameter("key", [g.CHUNK, g.SLOTS], f32, isOutput=False)
    wv_d = nc.declare_dram_parameter("wv", [g.CHUNK, g.SLOTS], f32, isOutput=False)
    iota_d = nc.declare_dram_parameter("iota", [g.CHUNK, g.J], f32, isOutput=False)
    Wt_d = nc.declare_dram_parameter("Wt", [IN, g.R * OUTD], f32, isOutput=False)
    root_d = nc.declare_dram_parameter("root", [IN, OUTD], f32, isOutput=False)
    bias_d = nc.declare_dram_parameter("bias", [1, OUTD], f32, isOutput=False)
    ones_d = nc.declare_dram_parameter("ones", [1, g.GRP], f32, isOutput=False)
    outT_d = nc.declare_dram_parameter("outT", [OUTD, g.NSH], f32, isOutput=True)

    with TileContext(nc) as tc:
        with (
            tc.tile_pool(name="consts", bufs=1) as consts,
            tc.tile_pool(name="meta", bufs=1) as meta,
            tc.tile_pool(name="gath", bufs=3) as gath,
            tc.tile_pool(name="sel", bufs=4) as sel,
            tc.tile_pool(name="zbig", bufs=2) as zbig_p,
            tc.tile_pool(name="outsb", bufs=1) as outsb_p,
            tc.tile_pool(name="psz", bufs=2, space="PSUM") as psz,
            tc.tile_pool(name="pso", bufs=2, space="PSUM") as pso,
        ):
            iota_t = consts.tile([g.CHUNK, g.J], f32)
            nc.sync.dma_start(out=iota_t[:], in_=iota_d[:])
            Wt_t = consts.tile([IN, g.R * OUTD], f32)
            nc.sync.dma_start(out=Wt_t[:], in_=Wt_d[:])
            root_t = consts.tile([IN, OUTD], f32)
            nc.sync.dma_start(out=root_t[:], in_=root_d[:])
            bias_t = consts.tile([1, OUTD], f32)
            nc.sync.dma_start(out=bias_t[:], in_=bias_d[:])
            ones_t = consts.tile([1, g.GRP], f32)
            nc.sync.dma_start(out=ones_t[:], in_=ones_d[:])
            xT_t = meta.tile([IN, g.NSH], f32)
            nc.sync.dma_start(out=xT_t[:], in_=xT_d[:])
            gidx_t = meta.tile([128, g.SLOTS * g.CHUNK // 16], i16)
            nc.sync.dma_start(out=gidx_t[:], in_=gidx_d[:])
            key_t = meta.tile([g.CHUNK, g.SLOTS], f32)
            nc.sync.dma_start(out=key_t[:], in_=key_d[:])
            wv_t = meta.tile([g.CHUNK, g.SLOTS], f32)
            nc.sync.dma_start(out=wv_t[:], in_=wv_d[:])
            outfull = outsb_p.tile([OUTD, g.NSH], f32)

            xg_tiles = [None] * g.NGATH

            def issue_gather(gi):
                tbl = xlo_d if gi * COLS_CALL < g.LO_COLS else xhi_d
                t = gath.tile([g.CHUNK, COLS_CALL * IN], f32, tag="xg")
                nc.gpsimd.dma_gather(
                    out_ap=t[:].rearrange("p (c f) -> p c f", c=COLS_CALL),
                    in_ap=tbl[:],
                    idxs_ap=gidx_t[:, gi * (NIDX_CALL // 16):(gi + 1) * (NIDX_CALL // 16)],
                    num_idxs=NIDX_CALL,
                    num_idxs_reg=NIDX_CALL,
                    elem_size=IN,
                    single_packet=False,
                )
                xg_tiles[gi] = t

            def slot_ref(s):
                gi, col = divmod(s, COLS_CALL)
                if xg_tiles[gi] is None:
                    issue_gather(gi)
                return xg_tiles[gi][:, col * IN:(col + 1) * IN]

            for g_i in range(g.NG):
                zb = zbig_p.tile([IN, g.WPG * g.J], f32)
                for w_i in range(g.WPG):
                    w_abs = g_i * g.WPG + w_i
                    zc = psz.tile([IN, g.J], f32, space="PSUM")
                    slots = ([g.slot(w_abs, 0, k) for k in range(g.C_LO)]
                             + [g.slot(w_abs, 1, k) for k in range(g.C_HI)])
                    for ki, s in enumerate(slots):
                        lhsT = slot_ref(s)
                        S = sel.tile([g.CHUNK, g.J], f32)
                        nc.vector.tensor_scalar(
                            out=S[:], in0=iota_t[:],
                            scalar1=key_t[:, s:s + 1],
                            scalar2=wv_t[:, s:s + 1],
                            op0=mybir.AluOpType.is_equal,
                            op1=mybir.AluOpType.mult,
                        )
                        nc.tensor.matmul(
                            out=zc[:], lhsT=lhsT, rhs=S[:],
                            start=(ki == 0), stop=(ki == len(slots) - 1),
                        )
                    nc.scalar.copy(out=zb[:, w_i * g.J:(w_i + 1) * g.J],
                                   in_=zc[:])
                po = pso.tile([OUTD, g.GRP], f32, space="PSUM")
                zb4 = zb[:].rearrange("p (w d r) -> p w d r",
                                      w=g.WPG, d=g.WIN, r=g.R)
                for r in range(g.R):
                    nc.tensor.matmul(
                        out=po[:],
                        lhsT=Wt_t[:, r * OUTD:(r + 1) * OUTD],
                        rhs=zb4[:, :, :, r],
                        start=(r == 0), stop=False,
                    )
                nc.tensor.matmul(
                    out=po[:], lhsT=root_t[:],
                    rhs=xT_t[:, g_i * g.GRP:(g_i + 1) * g.GRP],
                    start=False, stop=False,
                )
                nc.tensor.matmul(
                    out=po[:], lhsT=bias_t[:], rhs=ones_t[:],
                    start=False, stop=True,
                )
                nc.vector.tensor_copy(
                    out=outfull[:, g_i * g.GRP:(g_i + 1) * g.GRP], in_=po[:])
            nc.sync.dma_start(out=outT_d[:], in_=outfull[:])

    nc.finalize()
    return nc


def _get_program(g, IN, OUTD):
    k = (g.key(), IN, OUTD)
    if k not in _PROGRAM_CACHE:
        _PROGRAM_CACHE[k] = _build_program(g, IN, OUTD)
    return _PROGRAM_CACHE[k]


def _make_geom(n_nodes, n_rel, src, dst, et):
    """Pick slot counts from the data (uniform across cores)."""
    c_lo = c_hi = 1
    while True:
        g = _Geom(n_nodes, n_rel, c_lo, c_hi)
        try:
            for c in range(g.CORES):
                _prep_core(g, c, src, dst, et)
            return g
        except OverflowError as e:
            need_lo, need_hi = e.args[0]
            c_lo = max(c_lo, need_lo)
            c_hi = max(c_hi, need_hi)


# ---------------------------------------------------------------------------
# entry point
# ---------------------------------------------------------------------------

LAST_RESULTS = None


def kernel(x, edge_index, edge_type, edge_attr, basis, comp, root, bias,
           _trace=False):
    global LAST_RESULTS
    from concourse.bass_utils import run_bass_kernel_spmd

    x = np.asarray(x)
    edge_index = np.asarray(edge_index)
    edge_type = np.asarray(edge_type)
    n_nodes = x.shape[0]
    n_rel = np.asarray(comp).shape[0]

    g = _make_geom(n_nodes, n_rel, edge_index[0], edge_index[1], edge_type)
    in_maps = _host_prep(g, x, edge_index, edge_type, basis, comp, root, bias)

    IN = x.shape[1]
    OUTD = np.asarray(root).shape[1]
    nc = _get_program(g, IN, OUTD)
    res = run_bass_kernel_spmd(nc, in_maps, list(range(g.CORES)),
                               trace=_trace)
    LAST_RESULTS = res

    out = np.empty((n_nodes, OUTD), dtype=np.float32)
    for c in range(g.CORES):
        lo = c * g.NSH_REAL
        out[lo:lo + g.NSH_REAL] = res.results[c]["outT"][:, :g.NSH_REAL].T
    return out, np.asarray(edge_attr)


# revision 11
# speedup vs baseline: 1.3439x; 1.3439x over previous
"""RGCN basis-decomposition message-passing layer on 8 Trainium2 NeuronCores.

Reference semantics (per edge e: src -> dst with relation r, W[r] = comp[r] @ basis):
    msg[e]  = x[src[e]] @ W[r[e]] / count[dst[e], r[e]]
    agg     = segment_sum(msg, dst)
    out     = agg + x @ root + bias
    returns (out, edge_attr)          # edge_update is identity

Distribution: destination-sharded.  Core c owns dst nodes [c*N/8, (c+1)*N/8);
its edges are exactly the edges pointing into that range, so each core produces
a disjoint slice of the output and no collective is needed.

Device algorithm (aggregate-then-transform, matmul-based scatter):
  * edges sorted by local dst, bucketed into 64-dst windows; each window gets a
    fixed number of 128-edge chunk slots per source-half plane (the SPMD
    program is shared by all cores, so the slot grid is uniform; unused slots
    carry index 0 and weight 0 and contribute nothing).
  * source rows are fetched with gpsimd dma_gather (int16 indices, so x is
    split into a low and a high half table; slots are segregated by plane).
  * per chunk: a selection matrix S[e, j] = w_e * (iota[j] == dstloc_e*R + r_e)
    built by one dual-op tensor_scalar (j = 64 dst x 8 relations = 512), and
    one PE matmul accumulating Zt[i, j] += Xg^T @ S into a PSUM bank.
  * per 128-dst group (2 windows): outT[o, d] = sum_r W_r^T @ Zt_r
    + root^T @ xT + bias (rank-1), accumulated in one PSUM group.
  * output written transposed ([128 out x NSH dst]); host transposes back.

Counts (mean normalization) are index-only metadata; the host derives
w_e = 1/count[dst_e, r_e] from the same sort used to build the slot layout.
"""

import numpy as np

_PROGRAM_CACHE = {}

NIDX_CALL = 1024          # dma_gather indices per call (HW limit)
COLS_CALL = NIDX_CALL // 128


class _Geom:
    def __init__(self, n_nodes, n_rel, c_lo, c_hi, cores=8, win=64, grp=128,
                 chunk=128):
        assert n_nodes % cores == 0
        self.N = n_nodes
        self.R = n_rel
        self.CORES = cores
        self.WIN = win                      # dst nodes per window
        self.GRP = grp                      # dst nodes per group (psum out tile)
        self.CHUNK = chunk                  # edges per chunk (= K of matmul)
        self.J = win * n_rel                # one-hot width (<=512)
        assert self.J <= 512
        self.NSH_REAL = n_nodes // cores
        self.NG = -(-self.NSH_REAL // grp)
        self.NSH = self.NG * grp
        self.NW = self.NSH // win
        self.WPG = grp // win
        self.SPLIT = n_nodes // 2           # lo plane: src < SPLIT
        self.C_LO = c_lo                    # chunk slots per window, lo plane
        self.C_HI = c_hi
        # plane regions padded to whole gather calls
        self.LO_COLS = -(-self.NW * c_lo // COLS_CALL) * COLS_CALL
        self.HI_COLS = -(-self.NW * c_hi // COLS_CALL) * COLS_CALL
        self.SLOTS = self.LO_COLS + self.HI_COLS
        self.NGATH = self.SLOTS // COLS_CALL

    def slot(self, w, plane, k):
        if plane == 0:
            return w * self.C_LO + k
        return self.LO_COLS + w * self.C_HI + k

    def key(self):
        return (self.N, self.R, self.CORES, self.WIN, self.GRP, self.CHUNK,
                self.C_LO, self.C_HI)


# ---------------------------------------------------------------------------
# host-side sharding / metadata prep
# ---------------------------------------------------------------------------

def _prep_core(g, core, src, dst, et):
    lo = core * g.NSH_REAL
    hi = lo + g.NSH_REAL
    sel = np.nonzero((dst >= lo) & (dst < hi))[0]
    dloc = (dst[sel] - lo).astype(np.int64)
    s_src = src[sel].astype(np.int64)
    s_et = et[sel].astype(np.int64)

    plane = (s_src >= g.SPLIT).astype(np.int64)
    order = np.lexsort((dloc, plane, dloc // g.WIN))
    dloc, s_src, s_et, plane = (dloc[order], s_src[order], s_et[order],
                                plane[order])

    ckey = dloc * g.R + s_et
    cnt = np.bincount(ckey, minlength=g.NSH * g.R)
    w = 1.0 / cnt[ckey]

    wid = dloc // g.WIN
    # rank within (window, plane) run
    seg = wid * 2 + plane
    seg_starts = np.searchsorted(seg, np.arange(g.NW * 2))
    rank = np.arange(len(dloc)) - seg_starts[seg]
    kslot = rank // g.CHUNK
    c_lo_need = int(kslot[plane == 0].max()) + 1 if (plane == 0).any() else 0
    c_hi_need = int(kslot[plane == 1].max()) + 1 if (plane == 1).any() else 0
    if c_lo_need > g.C_LO or c_hi_need > g.C_HI:
        raise OverflowError((c_lo_need, c_hi_need))

    slot = np.where(plane == 0,
                    wid * g.C_LO + kslot,
                    g.LO_COLS + wid * g.C_HI + kslot)
    p = rank % g.CHUNK

    gidx = np.zeros((g.CHUNK, g.SLOTS), dtype=np.int16)
    key = np.full((g.CHUNK, g.SLOTS), 1000.0, dtype=np.float32)
    wv = np.zeros((g.CHUNK, g.SLOTS), dtype=np.float32)
    local_src = np.where(plane == 0, s_src, s_src - g.SPLIT)
    gidx[p, slot] = local_src.astype(np.int16)
    key[p, slot] = ((dloc - wid * g.WIN) * g.R + s_et).astype(np.float32)
    wv[p, slot] = w.astype(np.float32)

    # wrapped int16 index layout for dma_gather: stream position i ->
    # partition i % 16, column i // 16; replicated across the 8 q7 cores.
    flat = gidx.T.reshape(-1)                        # stream order (col-major)
    wrapped = flat.reshape(-1, 16).T                 # [16, SLOTS*8]
    gidx_dev = np.ascontiguousarray(np.tile(wrapped, (8, 1)))
    return gidx_dev, key, wv


def _host_prep(g, x, edge_index, edge_type, basis, comp, root, bias):
    src = np.asarray(edge_index[0])
    dst = np.asarray(edge_index[1])
    et = np.asarray(edge_type)
    x = np.ascontiguousarray(np.asarray(x, dtype=np.float32))
    IN = x.shape[1]
    OUTD = root.shape[1]

    W = np.einsum("rb,bio->rio", np.asarray(comp, np.float32),
                  np.asarray(basis, np.float32))
    Wt = np.ascontiguousarray(W.transpose(1, 0, 2).reshape(IN, g.R * OUTD))
    iota = np.ascontiguousarray(
        np.tile(np.arange(g.J, dtype=np.float16), (g.CHUNK, 1)))
    root_f = np.ascontiguousarray(np.asarray(root, np.float32))
    bias_f = np.ascontiguousarray(np.asarray(bias, np.float32).reshape(1, OUTD))
    ones = np.ones((1, g.GRP), dtype=np.float32)
    x_lo = np.ascontiguousarray(x[:g.SPLIT].astype(np.float16))
    x_hi = np.ascontiguousarray(x[g.SPLIT:].astype(np.float16))

    in_maps = []
    for c in range(g.CORES):
        gidx_dev, key, wv = _prep_core(g, c, src, dst, et)
        lo = c * g.NSH_REAL
        xT = np.zeros((IN, g.NSH), dtype=np.float32)
        xT[:, :g.NSH_REAL] = x[lo:lo + g.NSH_REAL].T
        in_maps.append({
            "x_lo": x_lo, "x_hi": x_hi,
            "xT": np.ascontiguousarray(xT),
            "gidx": gidx_dev, "key": key, "wv": wv,
            "iota": iota, "Wt": Wt, "root": root_f,
            "bias": bias_f, "ones": ones,
        })
    return in_maps


# ---------------------------------------------------------------------------
# device program
# ---------------------------------------------------------------------------

def _build_program(g, IN, OUTD):
    import concourse.bacc as bacc
    import concourse.mybir as mybir
    from concourse.tile import TileContext

    f32 = mybir.dt.float32
    f16 = mybir.dt.float16
    i16 = mybir.dt.int16
    nc = bacc.Bacc(None, target_bir_lowering=False)

    xlo_d = nc.declare_dram_parameter("x_lo", [g.SPLIT, IN], f16, isOutput=False)
    xhi_d = nc.declare_dram_parameter("x_hi", [g.N - g.SPLIT, IN], f16, isOutput=False)
    xT_d = nc.declare_dram_parameter("xT", [IN, g.NSH], f32, isOutput=False)
    gidx_d = nc.declare_dram_parameter("gidx", [128, g.SLOTS * g.CHUNK // 16], i16, isOutput=False)
    key_d = nc.declare_dram_parameter("key", [g.CHUNK, g.SLOTS], f32, isOutput=False)
    wv_d = nc.declare_dram_parameter("wv", [g.CHUNK, g.SLOTS], f32, isOutput=False)
    iota_d = nc.declare_dram_parameter("iota", [g.CHUNK, g.J], f16, isOutput=False)
    Wt_d = nc.declare_dram_parameter("Wt", [IN, g.R * OUTD], f32, isOutput=False)
    root_d = nc.declare_dram_parameter("root", [IN, OUTD], f32, isOutput=False)
    bias_d = nc.declare_dram_parameter("bias", [1, OUTD], f32, isOutput=False)
    ones_d = nc.declare_dram_parameter("ones", [1, g.GRP], f32, isOutput=False)
    outT_d = nc.declare_dram_parameter("outT", [OUTD, g.NSH], f32, isOutput=True)

    with TileContext(nc) as tc:
        with (
            tc.tile_pool(name="consts", bufs=1) as consts,
            tc.tile_pool(name="meta", bufs=1) as meta,
            tc.tile_pool(name="gath", bufs=3) as gath,
            tc.tile_pool(name="sel", bufs=4) as sel,
            tc.tile_pool(name="zbig", bufs=2) as zbig_p,
            tc.tile_pool(name="outsb", bufs=1) as outsb_p,
            tc.tile_pool(name="psz", bufs=2, space="PSUM") as psz,
            tc.tile_pool(name="pso", bufs=2, space="PSUM") as pso,
        ):
            iota_t = consts.tile([g.CHUNK, g.J], f16)
            nc.sync.dma_start(out=iota_t[:], in_=iota_d[:])
            Wt_t = consts.tile([IN, g.R * OUTD], f32)
            nc.sync.dma_start(out=Wt_t[:], in_=Wt_d[:])
            root_t = consts.tile([IN, OUTD], f32)
            nc.sync.dma_start(out=root_t[:], in_=root_d[:])
            bias_t = consts.tile([1, OUTD], f32)
            nc.sync.dma_start(out=bias_t[:], in_=bias_d[:])
            ones_t = consts.tile([1, g.GRP], f32)
            nc.sync.dma_start(out=ones_t[:], in_=ones_d[:])
            xT_t = meta.tile([IN, g.NSH], f32)
            nc.sync.dma_start(out=xT_t[:], in_=xT_d[:])
            gidx_t = meta.tile([128, g.SLOTS * g.CHUNK // 16], i16)
            nc.sync.dma_start(out=gidx_t[:], in_=gidx_d[:])
            key_t = meta.tile([g.CHUNK, g.SLOTS], f32)
            nc.sync.dma_start(out=key_t[:], in_=key_d[:])
            wv_t = meta.tile([g.CHUNK, g.SLOTS], f32)
            nc.sync.dma_start(out=wv_t[:], in_=wv_d[:])
            outfull = outsb_p.tile([OUTD, g.NSH], f32)

            xg_tiles = [None] * g.NGATH

            def issue_gather(gi):
                tbl = xlo_d if gi * COLS_CALL < g.LO_COLS else xhi_d
                t = gath.tile([g.CHUNK, COLS_CALL * IN], f16, tag="xg")
                nc.gpsimd.dma_gather(
                    out_ap=t[:].rearrange("p (c f) -> p c f", c=COLS_CALL),
                    in_ap=tbl[:],
                    idxs_ap=gidx_t[:, gi * (NIDX_CALL // 16):(gi + 1) * (NIDX_CALL // 16)],
                    num_idxs=NIDX_CALL,
                    num_idxs_reg=NIDX_CALL,
                    elem_size=IN,
                    single_packet=False,
                )
                xg_tiles[gi] = t

            def slot_ref(s):
                gi, col = divmod(s, COLS_CALL)
                if xg_tiles[gi] is None:
                    issue_gather(gi)
                return xg_tiles[gi][:, col * IN:(col + 1) * IN]

            for g_i in range(g.NG):
                zb = zbig_p.tile([IN, g.WPG * g.J], f32)
                for w_i in range(g.WPG):
                    w_abs = g_i * g.WPG + w_i
                    zc = psz.tile([IN, g.J], f32, space="PSUM")
                    slots = ([g.slot(w_abs, 0, k) for k in range(g.C_LO)]
                             + [g.slot(w_abs, 1, k) for k in range(g.C_HI)])
                    for ki, s in enumerate(slots):
                        lhsT = slot_ref(s)
                        S = sel.tile([g.CHUNK, g.J], f16)
                        nc.vector.tensor_scalar(
                            out=S[:], in0=iota_t[:],
                            scalar1=key_t[:, s:s + 1],
                            scalar2=wv_t[:, s:s + 1],
                            op0=mybir.AluOpType.is_equal,
                            op1=mybir.AluOpType.mult,
                        )
                        nc.tensor.matmul(
                            out=zc[:], lhsT=lhsT, rhs=S[:],
                            start=(ki == 0), stop=(ki == len(slots) - 1),
                        )
                    nc.scalar.copy(out=zb[:, w_i * g.J:(w_i + 1) * g.J],
                                   in_=zc[:])
                po = pso.tile([OUTD, g.GRP], f32, space="PSUM")
                zb4 = zb[:].rearrange("p (w d r) -> p w d r",
                                      w=g.WPG, d=g.WIN, r=g.R)
                for r in range(g.R):
                    nc.tensor.matmul(
                        out=po[:],
                        lhsT=Wt_t[:, r * OUTD:(r + 1) * OUTD],
                        rhs=zb4[:, :, :, r],
                        start=(r == 0), stop=False,
                    )
                nc.tensor.matmul(
                    out=po[:], lhsT=root_t[:],
                    rhs=xT_t[:, g_i * g.GRP:(g_i + 1) * g.GRP],
                    start=False, stop=False,
                )
                nc.tensor.matmul(
                    out=po[:], lhsT=bias_t[:], rhs=ones_t[:],
                    start=False, stop=True,
                )
                nc.vector.tensor_copy(
                    out=outfull[:, g_i * g.GRP:(g_i + 1) * g.GRP], in_=po[:])
            nc.sync.dma_start(out=outT_d[:], in_=outfull[:])

    nc.finalize()
    return nc


def _get_program(g, IN, OUTD):
    k = (g.key(), IN, OUTD)
    if k not in _PROGRAM_CACHE:
        _PROGRAM_CACHE[k] = _build_program(g, IN, OUTD)
    return _PROGRAM_CACHE[k]


def _make_geom(n_nodes, n_rel, src, dst, et):
    """Pick slot counts from the data (uniform across cores)."""
    c_lo = c_hi = 1
    while True:
        g = _Geom(n_nodes, n_rel, c_lo, c_hi)
        try:
            for c in range(g.CORES):
                _prep_core(g, c, src, dst, et)
            return g
        except OverflowError as e:
            need_lo, need_hi = e.args[0]
            c_lo = max(c_lo, need_lo)
            c_hi = max(c_hi, need_hi)


# ---------------------------------------------------------------------------
# entry point
# ---------------------------------------------------------------------------

LAST_RESULTS = None


def kernel(x, edge_index, edge_type, edge_attr, basis, comp, root, bias,
           _trace=False):
    global LAST_RESULTS
    from concourse.bass_utils import run_bass_kernel_spmd

    x = np.asarray(x)
    edge_index = np.asarray(edge_index)
    edge_type = np.asarray(edge_type)
    n_nodes = x.shape[0]
    n_rel = np.asarray(comp).shape[0]

    g = _make_geom(n_nodes, n_rel, edge_index[0], edge_index[1], edge_type)
    in_maps = _host_prep(g, x, edge_index, edge_type, basis, comp, root, bias)

    IN = x.shape[1]
    OUTD = np.asarray(root).shape[1]
    nc = _get_program(g, IN, OUTD)
    res = run_bass_kernel_spmd(nc, in_maps, list(range(g.CORES)),
                               trace=_trace)
    LAST_RESULTS = res

    out = np.empty((n_nodes, OUTD), dtype=np.float32)
    for c in range(g.CORES):
        lo = c * g.NSH_REAL
        out[lo:lo + g.NSH_REAL] = res.results[c]["outT"][:, :g.NSH_REAL].T
    return out, np.asarray(edge_attr)


# revision 13
# speedup vs baseline: 1.5498x; 1.1532x over previous
"""RGCN basis-decomposition message-passing layer on 8 Trainium2 NeuronCores.

Reference semantics (per edge e: src -> dst with relation r, W[r] = comp[r] @ basis):
    msg[e]  = x[src[e]] @ W[r[e]] / count[dst[e], r[e]]
    agg     = segment_sum(msg, dst)
    out     = agg + x @ root + bias
    returns (out, edge_attr)          # edge_update is identity

Distribution: destination-sharded.  Core c owns dst nodes [c*N/8, (c+1)*N/8);
its edges are exactly the edges pointing into that range, so each core produces
a disjoint slice of the output and no collective is needed.

Device algorithm (aggregate-then-transform, matmul-based scatter):
  * edges sorted by local dst, bucketed into 64-dst windows; each window gets a
    fixed number of 128-edge chunk slots per source-half plane (the SPMD
    program is shared by all cores, so the slot grid is uniform; unused slots
    carry index 0 and weight 0 and contribute nothing).
  * source rows are fetched with gpsimd dma_gather (int16 indices, so x is
    split into a low and a high half table; slots are segregated by plane).
  * per chunk: a selection matrix S[e, j] = w_e * (iota[j] == dstloc_e*R + r_e)
    built by one dual-op tensor_scalar (j = 64 dst x 8 relations = 512), and
    one PE matmul accumulating Zt[i, j] += Xg^T @ S into a PSUM bank.
  * per 128-dst group (2 windows): outT[o, d] = sum_r W_r^T @ Zt_r
    + root^T @ xT + bias (rank-1), accumulated in one PSUM group.
  * output written transposed ([128 out x NSH dst]); host transposes back.

Counts (mean normalization) are index-only metadata; the host derives
w_e = 1/count[dst_e, r_e] from the same sort used to build the slot layout.
"""

import numpy as np

_PROGRAM_CACHE = {}

NIDX_CALL = 1024          # dma_gather indices per call (HW limit)
COLS_CALL = NIDX_CALL // 128


class _Geom:
    def __init__(self, n_nodes, n_rel, c_lo, c_hi, cores=8, win=64, grp=128,
                 chunk=128):
        assert n_nodes % cores == 0
        self.N = n_nodes
        self.R = n_rel
        self.CORES = cores
        self.WIN = win                      # dst nodes per window
        self.GRP = grp                      # dst nodes per group (psum out tile)
        self.CHUNK = chunk                  # edges per chunk (= K of matmul)
        self.J = win * n_rel                # one-hot width (<=512)
        assert self.J <= 512
        self.NSH_REAL = n_nodes // cores
        self.NG = -(-self.NSH_REAL // grp)
        self.NSH = self.NG * grp
        self.NW = self.NSH // win
        self.WPG = grp // win
        self.SPLIT = n_nodes // 2           # lo plane: src < SPLIT
        self.C_LO = c_lo                    # chunk slots per window, lo plane
        self.C_HI = c_hi
        # plane regions padded to whole gather calls
        self.LO_COLS = -(-self.NW * c_lo // COLS_CALL) * COLS_CALL
        self.HI_COLS = -(-self.NW * c_hi // COLS_CALL) * COLS_CALL
        self.SLOTS = self.LO_COLS + self.HI_COLS
        self.NGATH = self.SLOTS // COLS_CALL

    def slot(self, w, plane, k):
        if plane == 0:
            return w * self.C_LO + k
        return self.LO_COLS + w * self.C_HI + k

    def key(self):
        return (self.N, self.R, self.CORES, self.WIN, self.GRP, self.CHUNK,
                self.C_LO, self.C_HI)


# ---------------------------------------------------------------------------
# host-side sharding / metadata prep
# ---------------------------------------------------------------------------

def _prep_core(g, core, src, dst, et):
    lo = core * g.NSH_REAL
    hi = lo + g.NSH_REAL
    sel = np.nonzero((dst >= lo) & (dst < hi))[0]
    dloc = (dst[sel] - lo).astype(np.int64)
    s_src = src[sel].astype(np.int64)
    s_et = et[sel].astype(np.int64)

    plane = (s_src >= g.SPLIT).astype(np.int64)
    order = np.lexsort((dloc, plane, dloc // g.WIN))
    dloc, s_src, s_et, plane = (dloc[order], s_src[order], s_et[order],
                                plane[order])

    ckey = dloc * g.R + s_et
    cnt = np.bincount(ckey, minlength=g.NSH * g.R)
    w = 1.0 / cnt[ckey]

    wid = dloc // g.WIN
    # rank within (window, plane) run
    seg = wid * 2 + plane
    seg_starts = np.searchsorted(seg, np.arange(g.NW * 2))
    rank = np.arange(len(dloc)) - seg_starts[seg]
    kslot = rank // g.CHUNK
    c_lo_need = int(kslot[plane == 0].max()) + 1 if (plane == 0).any() else 0
    c_hi_need = int(kslot[plane == 1].max()) + 1 if (plane == 1).any() else 0
    if c_lo_need > g.C_LO or c_hi_need > g.C_HI:
        raise OverflowError((c_lo_need, c_hi_need))

    slot = np.where(plane == 0,
                    wid * g.C_LO + kslot,
                    g.LO_COLS + wid * g.C_HI + kslot)
    p = rank % g.CHUNK

    gidx = np.zeros((g.CHUNK, g.SLOTS), dtype=np.int16)
    key = np.full((g.CHUNK, g.SLOTS), 1000.0, dtype=np.float32)
    wv = np.zeros((g.CHUNK, g.SLOTS), dtype=np.float32)
    local_src = np.where(plane == 0, s_src, s_src - g.SPLIT)
    gidx[p, slot] = local_src.astype(np.int16)
    key[p, slot] = ((dloc - wid * g.WIN) * g.R + s_et).astype(np.float32)
    wv[p, slot] = w.astype(np.float32)

    # wrapped int16 index layout for dma_gather: stream position i ->
    # partition i % 16, column i // 16; replicated across the 8 q7 cores.
    flat = gidx.T.reshape(-1)                        # stream order (col-major)
    wrapped = flat.reshape(-1, 16).T                 # [16, SLOTS*8]
    gidx_dev = np.ascontiguousarray(np.tile(wrapped, (8, 1)))
    return gidx_dev, key, wv


def _host_prep(g, x, edge_index, edge_type, basis, comp, root, bias):
    src = np.asarray(edge_index[0])
    dst = np.asarray(edge_index[1])
    et = np.asarray(edge_type)
    x = np.ascontiguousarray(np.asarray(x, dtype=np.float32))
    IN = x.shape[1]
    OUTD = root.shape[1]

    W = np.einsum("rb,bio->rio", np.asarray(comp, np.float32),
                  np.asarray(basis, np.float32))
    Wt = np.ascontiguousarray(W.transpose(1, 0, 2).reshape(IN, g.R * OUTD))
    iota = np.ascontiguousarray(
        np.tile(np.arange(g.J, dtype=np.float16), (g.CHUNK, 1)))
    root_f = np.ascontiguousarray(np.asarray(root, np.float32))
    bias_f = np.ascontiguousarray(np.asarray(bias, np.float32).reshape(1, OUTD))
    ones = np.ones((1, g.GRP), dtype=np.float32)
    x_lo = np.ascontiguousarray(x[:g.SPLIT].astype(np.float16))
    x_hi = np.ascontiguousarray(x[g.SPLIT:].astype(np.float16))

    in_maps = []
    for c in range(g.CORES):
        gidx_dev, key, wv = _prep_core(g, c, src, dst, et)
        lo = c * g.NSH_REAL
        xT = np.zeros((IN, g.NSH), dtype=np.float32)
        xT[:, :g.NSH_REAL] = x[lo:lo + g.NSH_REAL].T
        in_maps.append({
            "x_lo": x_lo, "x_hi": x_hi,
            "xT": np.ascontiguousarray(xT),
            "gidx": gidx_dev, "key": key, "wv": wv,
            "iota": iota, "Wt": Wt, "root": root_f,
            "bias": bias_f, "ones": ones,
        })
    return in_maps


# ---------------------------------------------------------------------------
# device program
# ---------------------------------------------------------------------------

def _build_program(g, IN, OUTD):
    import concourse.bacc as bacc
    import concourse.mybir as mybir
    from concourse.tile import TileContext

    f32 = mybir.dt.float32
    f16 = mybir.dt.float16
    i16 = mybir.dt.int16
    nc = bacc.Bacc(None, target_bir_lowering=False)

    xlo_d = nc.declare_dram_parameter("x_lo", [g.SPLIT, IN], f16, isOutput=False)
    xhi_d = nc.declare_dram_parameter("x_hi", [g.N - g.SPLIT, IN], f16, isOutput=False)
    xT_d = nc.declare_dram_parameter("xT", [IN, g.NSH], f32, isOutput=False)
    gidx_d = nc.declare_dram_parameter("gidx", [128, g.SLOTS * g.CHUNK // 16], i16, isOutput=False)
    key_d = nc.declare_dram_parameter("key", [g.CHUNK, g.SLOTS], f32, isOutput=False)
    wv_d = nc.declare_dram_parameter("wv", [g.CHUNK, g.SLOTS], f32, isOutput=False)
    iota_d = nc.declare_dram_parameter("iota", [g.CHUNK, g.J], f16, isOutput=False)
    Wt_d = nc.declare_dram_parameter("Wt", [IN, g.R * OUTD], f32, isOutput=False)
    root_d = nc.declare_dram_parameter("root", [IN, OUTD], f32, isOutput=False)
    bias_d = nc.declare_dram_parameter("bias", [1, OUTD], f32, isOutput=False)
    ones_d = nc.declare_dram_parameter("ones", [1, g.GRP], f32, isOutput=False)
    outT_d = nc.declare_dram_parameter("outT", [OUTD, g.NSH], f32, isOutput=True)

    with TileContext(nc) as tc:
        with (
            tc.tile_pool(name="consts", bufs=1) as consts,
            tc.tile_pool(name="meta", bufs=1) as meta,
            tc.tile_pool(name="gath", bufs=8) as gath,
            tc.tile_pool(name="sel", bufs=8) as sel,
            tc.tile_pool(name="zbig", bufs=2) as zbig_p,
            tc.tile_pool(name="outsb", bufs=1) as outsb_p,
            tc.tile_pool(name="psz", bufs=3, space="PSUM") as psz,
            tc.tile_pool(name="pso", bufs=2, space="PSUM") as pso,
        ):
            iota_t = consts.tile([g.CHUNK, g.J], f16)
            nc.sync.dma_start(out=iota_t[:], in_=iota_d[:])
            Wt_t = consts.tile([IN, g.R * OUTD], f32)
            nc.sync.dma_start(out=Wt_t[:], in_=Wt_d[:])
            root_t = consts.tile([IN, OUTD], f32)
            nc.sync.dma_start(out=root_t[:], in_=root_d[:])
            bias_t = consts.tile([1, OUTD], f32)
            nc.sync.dma_start(out=bias_t[:], in_=bias_d[:])
            ones_t = consts.tile([1, g.GRP], f32)
            nc.sync.dma_start(out=ones_t[:], in_=ones_d[:])
            xT_t = meta.tile([IN, g.NSH], f32)
            nc.sync.dma_start(out=xT_t[:], in_=xT_d[:])
            gidx_t = meta.tile([128, g.SLOTS * g.CHUNK // 16], i16)
            nc.sync.dma_start(out=gidx_t[:], in_=gidx_d[:])
            key_t = meta.tile([g.CHUNK, g.SLOTS], f32)
            nc.sync.dma_start(out=key_t[:], in_=key_d[:])
            wv_t = meta.tile([g.CHUNK, g.SLOTS], f32)
            nc.sync.dma_start(out=wv_t[:], in_=wv_d[:])
            outfull = outsb_p.tile([OUTD, g.NSH], f32)

            xg_tiles = [None] * g.NGATH

            def issue_gather(gi):
                tbl = xlo_d if gi * COLS_CALL < g.LO_COLS else xhi_d
                t = gath.tile([g.CHUNK, COLS_CALL * IN], f16, tag="xg")
                nc.gpsimd.dma_gather(
                    out_ap=t[:].rearrange("p (c f) -> p c f", c=COLS_CALL),
                    in_ap=tbl[:],
                    idxs_ap=gidx_t[:, gi * (NIDX_CALL // 16):(gi + 1) * (NIDX_CALL // 16)],
                    num_idxs=NIDX_CALL,
                    num_idxs_reg=NIDX_CALL,
                    elem_size=IN,
                    single_packet=False,
                )
                xg_tiles[gi] = t

            def slot_ref(s):
                gi, col = divmod(s, COLS_CALL)
                if xg_tiles[gi] is None:
                    issue_gather(gi)
                return xg_tiles[gi][:, col * IN:(col + 1) * IN]

            for g_i in range(g.NG):
                zb = zbig_p.tile([IN, g.WPG * g.J], f32)
                for w_i in range(g.WPG):
                    w_abs = g_i * g.WPG + w_i
                    zc = psz.tile([IN, g.J], f32, space="PSUM")
                    slots = ([g.slot(w_abs, 0, k) for k in range(g.C_LO)]
                             + [g.slot(w_abs, 1, k) for k in range(g.C_HI)])
                    for ki, s in enumerate(slots):
                        lhsT = slot_ref(s)
                        S = sel.tile([g.CHUNK, g.J], f16)
                        nc.vector.tensor_scalar(
                            out=S[:], in0=iota_t[:],
                            scalar1=key_t[:, s:s + 1],
                            scalar2=wv_t[:, s:s + 1],
                            op0=mybir.AluOpType.is_equal,
                            op1=mybir.AluOpType.mult,
                        )
                        nc.tensor.matmul(
                            out=zc[:], lhsT=lhsT, rhs=S[:],
                            start=(ki == 0), stop=(ki == len(slots) - 1),
                        )
                    nc.scalar.copy(out=zb[:, w_i * g.J:(w_i + 1) * g.J],
                                   in_=zc[:])
                po = pso.tile([OUTD, g.GRP], f32, space="PSUM")
                zb4 = zb[:].rearrange("p (w d r) -> p w d r",
                                      w=g.WPG, d=g.WIN, r=g.R)
                for r in range(g.R):
                    nc.tensor.matmul(
                        out=po[:],
                        lhsT=Wt_t[:, r * OUTD:(r + 1) * OUTD],
                        rhs=zb4[:, :, :, r],
                        start=(r == 0), stop=False,
                    )
                nc.tensor.matmul(
                    out=po[:], lhsT=root_t[:],
                    rhs=xT_t[:, g_i * g.GRP:(g_i + 1) * g.GRP],
                    start=False, stop=False,
                )
                nc.tensor.matmul(
                    out=po[:], lhsT=bias_t[:], rhs=ones_t[:],
                    start=False, stop=True,
                )
                nc.vector.tensor_copy(
                    out=outfull[:, g_i * g.GRP:(g_i + 1) * g.GRP], in_=po[:])
            nc.sync.dma_start(out=outT_d[:], in_=outfull[:])

    nc.finalize()
    return nc


def _get_program(g, IN, OUTD):
    k = (g.key(), IN, OUTD)
    if k not in _PROGRAM_CACHE:
        _PROGRAM_CACHE[k] = _build_program(g, IN, OUTD)
    return _PROGRAM_CACHE[k]


def _make_geom(n_nodes, n_rel, src, dst, et):
    """Pick slot counts from the data (uniform across cores)."""
    c_lo = c_hi = 1
    while True:
        g = _Geom(n_nodes, n_rel, c_lo, c_hi)
        try:
            for c in range(g.CORES):
                _prep_core(g, c, src, dst, et)
            return g
        except OverflowError as e:
            need_lo, need_hi = e.args[0]
            c_lo = max(c_lo, need_lo)
            c_hi = max(c_hi, need_hi)


# ---------------------------------------------------------------------------
# entry point
# ---------------------------------------------------------------------------

LAST_RESULTS = None


def kernel(x, edge_index, edge_type, edge_attr, basis, comp, root, bias,
           _trace=False):
    global LAST_RESULTS
    from concourse.bass_utils import run_bass_kernel_spmd

    x = np.asarray(x)
    edge_index = np.asarray(edge_index)
    edge_type = np.asarray(edge_type)
    n_nodes = x.shape[0]
    n_rel = np.asarray(comp).shape[0]

    g = _make_geom(n_nodes, n_rel, edge_index[0], edge_index[1], edge_type)
    in_maps = _host_prep(g, x, edge_index, edge_type, basis, comp, root, bias)

    IN = x.shape[1]
    OUTD = np.asarray(root).shape[1]
    nc = _get_program(g, IN, OUTD)
    res = run_bass_kernel_spmd(nc, in_maps, list(range(g.CORES)),
                               trace=_trace)
    LAST_RESULTS = res

    out = np.empty((n_nodes, OUTD), dtype=np.float32)
    for c in range(g.CORES):
        lo = c * g.NSH_REAL
        out[lo:lo + g.NSH_REAL] = res.results[c]["outT"][:, :g.NSH_REAL].T
    return out, np.asarray(edge_attr)
